# revision 22
# baseline (speedup 1.0000x reference)
"""Trainium2 Bass kernel for nn_AutoRegressiveDecoderLayer.

One transformer decoder step (self-attn with KV cache + masked cross-attn +
MLP, each followed by LayerNorm) over bsz=1024, dim=128, 8 heads.

Strategy: pure data parallel over the batch — 8 NeuronCores, 128 batch
elements each.  Per core everything is expressed on 128-partition tiles.

Key layout decisions (v2):
- The host pre-transposes K to K^T [dim, keys] and pre-casts K and V to
  bf16, stored d-outermost so each per-group DMA moves 8 KB contiguous per
  partition (128 descriptors).  No on-chip K transposes at all.
- V is host-row-permuted so a natural chunked load [p, (c d)] puts key
  t = 128c + p on partition p of chunk c — exactly matching the column
  order of A^T chunks (which are plain PE transposes of the score matrix).
- Self-attn is padded to 512 keys; the fresh key's K^T column is copied
  into kt slot 511 on-chip (plain [128,1] copy in dT layout), so scores
  need no special-casing.  The fresh V row cannot be placed on partition
  127 cheaply, so its AV contribution is added via a tiny per-element
  head-select matmul (anew) + extraction fixup, as in v1.
- Cross-attn is padded to 1024 keys; pad columns get -1e9 in the host
  mask so exp() zeroes them.
- Scores: per batch element, Q-block [128,8] stationary x K^T moving.
  4 elements share a PSUM bank at 32-partition offsets (tile_position).
- Softmax: exp (no max-sub; |scores| small) with fused row-sum, scale.
- A^T via PE transpose per 128-chunk, PSUM->SBUF copy casts to bf16.
- AV: per element per chunk, V-chunk [128,128] bf16 stationary x A^T
  slice [128,8] moving, accumulated in a shared PSUM bank; extracted with
  a block-diagonal mask multiply + reduce into dT layout.
- LayerNorm transposes to batch-major, normalizes with per-partition
  scalars, applies gamma/beta via PE-broadcast tiles, transposes back.
"""

import os

import numpy as np
import ml_dtypes

import concourse.bass as bass
import concourse.bacc as bacc
import concourse.tile as tile
from concourse import mybir

F32 = mybir.dt.float32
BF16 = mybir.dt.bfloat16
AFT = mybir.ActivationFunctionType
AX = mybir.AxisListType
ALU = mybir.AluOpType

DIM = 128
NB_HEADS = 8
DH = DIM // NB_HEADS
N_CORES = 8
BSZ = 1024
NK = 1000   # cross-attention keys (valid)
NKP = 1024  # cross-attention keys (padded)
TP = 511    # self-attn KV cache length (previous)
TS = 512    # self-attn padded length (incl fresh key at slot 511)
LN_EPS = 1e-5

_WNAMES = ["Wq_sa", "Wk_sa", "Wv_sa", "W0_sa", "Wq_a", "W0_a", "W1", "W2"]
_BNAMES = ["bq_sa", "bk_sa", "bv_sa", "b0_sa", "bq_a", "b0_a", "b1", "b2"]
_GNAMES = ["g_sa", "g_a", "g_mlp"]
_BENAMES = ["be_sa", "be_a", "be_mlp"]


def _bc(ap, idx, count):
    """Insert a step-0 (broadcast) dim of `count` at position idx."""
    new = [list(p) for p in ap.ap]
    new.insert(idx, [0, count])
    return bass.AP(ap.tensor, ap.offset, new)


def build_nc(B, reps=1):
    """Build the Bass program for one core processing B batch elements."""
    nc = bacc.Bacc("TRN2", target_bir_lowering=False, debug=False)

    def dpi(name, shape, dt=F32):
        return nc.declare_dram_parameter(name, list(shape), dt, isOutput=False).ap()

    d = {}
    d["h_t"] = dpi("h_t", (B, DIM))
    # host-prepared: K^T d-outermost, V row-permuted p-outermost, both bf16
    d["KT_att"] = dpi("KT_att", (DIM, B, NKP), BF16)
    d["Vp_att"] = dpi("Vp_att", (DIM, B, NKP), BF16)   # [p, b, (c d)]
    d["KT_sa"] = dpi("KT_sa", (DIM, B, TS), BF16)
    d["Vp_sa"] = dpi("Vp_sa", (DIM, B, TS), BF16)      # [p, b, (c d)]
    d["maskf"] = dpi("maskf", (B, NKP), BF16)
    for w in _WNAMES:
        d[w] = dpi(w, (DIM, DIM))
    for b in _BNAMES:
        d[b] = dpi(b, (DIM, 1))
    for g in _GNAMES + _BENAMES:
        d[g] = dpi(g, (1, DIM))
    d["ident"] = dpi("ident", (128, 128))
    d["ident_bf"] = dpi("ident_bf", (128, 128), BF16)
    d["seg8"] = dpi("seg8", (128, 8))
    d["segT8"] = dpi("segT8", (128, 128))
    d["E4"] = dpi("E4", (4, 128), BF16)
    out_h = nc.declare_dram_parameter("out", [B, DIM], F32, isOutput=True).ap()

    with tile.TileContext(nc) as tc:
        for _ in range(reps):
            _emit(nc, tc, d, out_h, B)
    nc.compile()
    return nc


def _emit(nc, tc, d, out_h, B):
    """Emit the full per-core program, pipelined in sub-batches of 64."""
    assert B % 4 == 0
    from contextlib import ExitStack

    SB = min(64, B)
    assert B % SB == 0

    with ExitStack() as ctx:
        # ---------------- pools ----------------
        pers = ctx.enter_context(tc.tile_pool(name="pers", bufs=1))
        sm = ctx.enter_context(tc.tile_pool(name="sm", bufs=3))
        # K/V streaming pool: one buf holds a whole group-of-4's K^T+V for
        # both attention stages; deep buffering keeps DMA prefetch ahead.
        p_kv = ctx.enter_context(tc.tile_pool(name="kv", bufs=4))
        p_a = ctx.enter_context(tc.tile_pool(name="pa", bufs=2))
        p_at = ctx.enter_context(tc.tile_pool(name="pat", bufs=2))
        p_x = ctx.enter_context(tc.tile_pool(name="px", bufs=2))
        p_mk = ctx.enter_context(tc.tile_pool(name="pmk", bufs=3))
        # PSUM: S_self(1) + S_cross(2) + tp(3) + av(1) = 7 banks
        p_ss = ctx.enter_context(tc.tile_pool(name="pss", bufs=1, space="PSUM"))
        p_sc = ctx.enter_context(tc.tile_pool(name="psc", bufs=1, space="PSUM"))
        p_tp = ctx.enter_context(tc.tile_pool(name="ptp", bufs=3, space="PSUM"))
        p_av = ctx.enter_context(tc.tile_pool(name="pav", bufs=1, space="PSUM"))
        pools = dict(p_kv=p_kv, p_a=p_a, p_at=p_at, p_x=p_x, p_mk=p_mk,
                     p_ss=p_ss, p_sc=p_sc, p_tp=p_tp, p_av=p_av, sm=sm)

        def pt(pool, shape, dtype, tag):
            return pool.tile(list(shape), dtype, tag=tag, name=tag)

        # ---------------- constants / weights ----------------
        ident = pt(pers, (128, 128), F32, "ident")
        nc.sync.dma_start(ident[:], d["ident"])
        ident_bf = pt(pers, (128, 128), BF16, "ident_bf")
        nc.sync.dma_start(ident_bf[:], d["ident_bf"])
        seg8 = pt(pers, (128, 8), F32, "seg8")
        nc.sync.dma_start(seg8[:], d["seg8"])
        segT8 = pt(pers, (128, 128), F32, "segT8")
        nc.sync.dma_start(segT8[:], d["segT8"])
        E4 = pt(pers, (4, 128), BF16, "E4")
        nc.sync.dma_start(E4[:], d["E4"])
        zeros4 = pt(pers, (4, 512), BF16, "zeros4")
        nc.vector.memset(zeros4[:], 0.0)

        W = {}
        for w in _WNAMES:
            W[w] = pt(pers, (128, 128), F32, w)
            nc.sync.dma_start(W[w][:], d[w])
        Bi = {}
        for b in _BNAMES:
            Bi[b] = pt(pers, (128, 1), F32, b)
            nc.sync.dma_start(Bi[b][:], d[b])

        # gamma/beta broadcast tiles: ones[1,B].T @ row[1,128] -> [B,128]
        ones1 = pt(pers, (1, B), F32, "ones1")
        nc.vector.memset(ones1[:], 1.0)
        gb_rep = {}
        for nm in _GNAMES + _BENAMES:
            row = pt(pers, (1, 128), F32, "row_" + nm)
            nc.sync.dma_start(row[:], d[nm])
            ps = pt(p_tp, (B, 128), F32, "tp")
            nc.tensor.matmul(ps[:], ones1[:], row[:], start=True, stop=True)
            rep = pt(pers, (B, 128), F32, "rep_" + nm)
            nc.scalar.copy(rep[:], ps[:])
            gb_rep[nm] = rep

        # ---------------- h_t and qkv projections (all B) ----------------
        h_nat = pt(pers, (B, 128), F32, "h_nat")
        nc.sync.dma_start(h_nat[:], d["h_t"])
        hT = _transpose_to(nc, p_tp, pers, h_nat[:], ident, (128, B), "hT")

        def linear(rhs, wname, bname, out_pool, out_tag, func=AFT.Identity,
                   dtype=F32):
            w_ = rhs.free_size()
            ps = pt(p_tp, (128, w_), F32, "tp")
            nc.tensor.matmul(ps[:], W[wname][:], rhs, start=True, stop=True)
            out = pt(out_pool, (128, w_), dtype, out_tag)
            nc.scalar.activation(out[:], ps[:], func, bias=Bi[bname][:])
            return out

        q_saT = linear(hT[:], "Wq_sa", "bq_sa", pers, "q_saT")
        k_saT_bf = linear(hT[:], "Wk_sa", "bk_sa", pers, "k_saT_bf", dtype=BF16)
        v_saT = linear(hT[:], "Wv_sa", "bv_sa", pers, "v_saT")

        def q_blk(qT_ap, out, col0, nb):
            ov = out[:, 8 * col0:8 * (col0 + nb)].rearrange(
                "p (b h) -> p b h", h=8)
            qv = _bc(qT_ap, 2, 8)
            sv = _bc(seg8[:], 1, nb)
            nc.vector.tensor_mul(ov, qv, sv)

        Qb_sa = pt(pers, (128, 8 * B), BF16, "Qb_sa")
        q_blk(q_saT[:], Qb_sa, 0, B)

        # ---------------- pipelined halves ----------------
        for s0 in range(0, B, SB):
            sl = slice(s0, s0 + SB)
            attn1 = pt(sm, (128, SB), F32, "attn1")
            _attention(
                nc, tc, pools, b_lo=s0, nb=SB,
                KTsrc=d["KT_sa"], Vsrc=d["Vp_sa"], ncols=TS,
                Qb=Qb_sa, maskf=None, E4=E4, zeros4=zeros4,
                ident=ident, ident_bf=ident_bf, seg8=seg8, segT8=segT8,
                new_kv=(k_saT_bf, v_saT), attn_out=attn1[:], tagp="s",
            )
            t0 = linear(attn1[:], "W0_sa", "b0_sa", sm, "t0")
            h1T = pt(sm, (128, SB), F32, "h1T")
            nc.vector.tensor_add(h1T[:], t0[:], hT[:, sl])
            h1nT = _layernorm(nc, tc, p_tp, sm, h1T[:], ident,
                              gb_rep["g_sa"], gb_rep["be_sa"], s0, SB,
                              "h1n", out_T=True)
            q_aT = linear(h1nT[:], "Wq_a", "bq_a", sm, "q_aT")
            Qb_a = pt(sm, (128, 8 * SB), BF16, "Qb_a")
            q_blk(q_aT[:], Qb_a, 0, SB)
            attn2 = pt(sm, (128, SB), F32, "attn2")
            _attention(
                nc, tc, pools, b_lo=s0, nb=SB,
                KTsrc=d["KT_att"], Vsrc=d["Vp_att"], ncols=NKP,
                Qb=Qb_a, maskf=d["maskf"], E4=E4, zeros4=zeros4,
                ident=ident, ident_bf=ident_bf, seg8=seg8, segT8=segT8,
                new_kv=None, attn_out=attn2[:], tagp="c", qb_lo=s0,
            )
            t1 = linear(attn2[:], "W0_a", "b0_a", sm, "t1")
            h2T = pt(sm, (128, SB), F32, "h2T")
            nc.vector.tensor_add(h2T[:], t1[:], h1nT[:])
            h2nT = _layernorm(nc, tc, p_tp, sm, h2T[:], ident,
                              gb_rep["g_a"], gb_rep["be_a"], s0, SB,
                              "h2n", out_T=True)
            mT = linear(h2nT[:], "W1", "b1", sm, "mT", func=AFT.Relu)
            t2 = linear(mT[:], "W2", "b2", sm, "t2")
            h3T = pt(sm, (128, SB), F32, "h3T")
            nc.vector.tensor_add(h3T[:], t2[:], h2nT[:])
            out_nat = _layernorm(nc, tc, p_tp, sm, h3T[:], ident,
                                 gb_rep["g_mlp"], gb_rep["be_mlp"], s0, SB,
                                 "h3n", out_T=False)
            nc.sync.dma_start(out_h[sl, :], out_nat[:])


def _transpose_to(nc, p_ps, pool, in_ap, ident, out_shape, tag):
    """PE transpose (fp32) + ACT copy to a new sbuf tile."""
    P, F = in_ap.partition_size(), in_ap.free_size()
    ps = p_ps.tile([F, P], F32, tag="tp", name="tp")
    nc.tensor.matmul(ps[:], in_ap, ident[0:P, 0:P], is_transpose=True,
                     start=True, stop=True)
    out = pool.tile(list(out_shape), F32, tag=tag, name=tag)
    nc.scalar.copy(out[:], ps[:])
    return out


def _layernorm(nc, tc, p_tp, sm, xT_ap, ident, g_rep, be_rep, s0, SB, tag,
               out_T):
    """LayerNorm over dim for xT [128(dim), SB]; batch rows s0..s0+SB."""
    nat = _transpose_to(nc, p_tp, sm, xT_ap, ident, (SB, 128), tag + "_nat")
    negmu = sm.tile([SB, 1], F32, tag=tag + "_negmu", name=tag + "_negmu")
    nc.vector.tensor_reduce(negmu[:], nat[:], axis=AX.X, op=ALU.add,
                            negate=True)
    nc.vector.tensor_scalar_mul(negmu[:], negmu[:], 1.0 / DIM)
    cent = sm.tile([SB, 128], F32, tag=tag + "_cent", name=tag + "_cent")
    nc.vector.tensor_scalar_add(cent[:], nat[:], negmu[:])
    sq = sm.tile([SB, 128], F32, tag=tag + "_sq", name=tag + "_sq")
    ssq = sm.tile([SB, 1], F32, tag=tag + "_ssq", name=tag + "_ssq")
    nc.scalar.activation(sq[:], cent[:], AFT.Square, accum_out=ssq[:])
    var = sm.tile([SB, 1], F32, tag=tag + "_var", name=tag + "_var")
    nc.vector.tensor_scalar(var[:], ssq[:], 1.0 / DIM, LN_EPS,
                            op0=ALU.mult, op1=ALU.add)
    sd = sm.tile([SB, 1], F32, tag=tag + "_sd", name=tag + "_sd")
    nc.scalar.activation(sd[:], var[:], AFT.Sqrt)
    rstd = sm.tile([SB, 1], F32, tag=tag + "_rstd", name=tag + "_rstd")
    nc.vector.reciprocal(rstd[:], sd[:])
    nc.vector.tensor_scalar_mul(cent[:], cent[:], rstd[:])
    nc.vector.tensor_mul(cent[:], cent[:], g_rep[0:SB, :])
    nc.vector.tensor_add(cent[:], cent[:], be_rep[0:SB, :])
    if not out_T:
        return cent
    return _transpose_to(nc, p_tp, sm, cent[:], ident, (128, SB), tag + "_T")


def _attention(nc, tc, pools, *, b_lo, nb, KTsrc, Vsrc, ncols, Qb,
               maskf, E4, zeros4, ident, ident_bf, seg8, segT8, new_kv,
               attn_out, tagp, qb_lo=None):
    """One attention stage for batch rows [b_lo, b_lo+nb), nb <= 64.

    KTsrc: dram AP [128(d), B, ncols] bf16 — pre-transposed K.
    Vsrc:  dram AP [128(p), B, ncols] bf16 — V rows permuted so that
           chunk c, partition p holds key t = 128c + p.
    Scores for 4 batch elements share one PSUM tile at 32-partition
    offsets; softmax is exp (no max-sub) + fused row-sum + scale.
    new_kv: (k_T_bf16 [128,B], v_T_f32 [128,B]) — fresh self-attn K/V.
    The K column is copied into kt slot ncols-1 (so scores include it);
    the V contribution is added via a per-element head-select matmul
    (anew) + extraction fixup, since V slot (127, last) can't be filled
    by a cheap same-partition copy.  attn_out [128, nb].
    """
    assert nb <= 64 and nb % 4 == 0
    if qb_lo is None:
        qb_lo = 0
    nch = ncols // 128
    banks = [(s, min(512, ncols - s)) for s in range(0, ncols, 512)]

    p_kv = pools["p_kv"]
    p_a = pools["p_a"]
    p_at = pools["p_at"]
    p_x = pools["p_x"]
    p_mk = pools["p_mk"]
    p_sc = pools["p_ss"] if ncols <= 512 else pools["p_sc"]
    p_tp = pools["p_tp"]
    p_av = pools["p_av"]
    sm = pools["sm"]
    stag = "S_s" if ncols <= 512 else "S_c"

    av_ps = p_av.tile([128, nb * 8], F32, tag="av", name="av")
    anew_ps = None
    if new_kv is not None:
        anew_ps = p_av.tile([128, nb], F32, tag="anew", name="anew")
    for g in range(nb // 4):
        gb = b_lo + 4 * g
        # --- stream K^T and V for the group of 4 (one DMA each) ---
        kt = p_kv.tile([128, 4 * ncols], BF16, tag="kt" + tagp, name="kt" + tagp)
        # split HWDGE issue: cross-K on SP, self-K on ACT
        keng = nc.sync if maskf is not None else nc.scalar
        for _x in range(1 + int(os.environ.get("PROBE_DMA", "0"))):
            keng.dma_start(
                kt[:].rearrange("d (b t) -> d b t", b=4),
                KTsrc[:, gb:gb + 4, :])
        vt = p_kv.tile([128, 4 * ncols], BF16, tag="vt" + tagp, name="vt" + tagp)
        for _x in range(1 + int(os.environ.get("PROBE_VDMA", "0"))):
            nc.gpsimd.dma_start(
                vt[:].rearrange("p (b t) -> p b t", b=4),
                Vsrc[:, gb:gb + 4, :])
        if new_kv is not None:
            k_T_bf = new_kv[0]
            for j in range(4):
                b = gb + j
                nc.vector.tensor_copy(
                    kt[:, j * ncols + ncols - 1:j * ncols + ncols],
                    k_T_bf[:, b:b + 1])

        S = p_sc.tile([128, ncols], F32, tag=stag, name=stag)
        # --- init: mask rows (cross) or zeros (self), one MM per bank ---
        if maskf is not None:
            mk = p_mk.tile([4, ncols], BF16, tag="mk", name="mk")
            nc.scalar.dma_start(mk[:], maskf[gb:gb + 4, :])
            for (s0_, w) in banks:
                nc.tensor.matmul(S[:, s0_:s0_ + w], E4[:], mk[:, s0_:s0_ + w],
                                 start=True, stop=True, skip_group_check=True)
        else:
            for (s0_, w) in banks:
                nc.tensor.matmul(S[:, s0_:s0_ + w], E4[:], zeros4[:, 0:w],
                                 start=True, stop=True, skip_group_check=True)
        # --- scores ---
        for j in range(4):
            b = gb + j
            qb = Qb[:, 8 * (b - qb_lo):8 * (b - qb_lo) + 8]
            row = S[32 * j:32 * j + 8, :]
            for (s0_, w) in banks:
                nc.tensor.matmul(row[:, s0_:s0_ + w],
                                 qb, kt[:, j * ncols + s0_:j * ncols + s0_ + w],
                                 start=False, stop=True,
                                 tile_position=(0, 32 * j),
                                 skip_group_check=True)
        # --- softmax: exp + fused row-sum, then reciprocal scale ---
        A = p_a.tile([128, ncols], F32, tag="A", name="A")
        sums = sm.tile([128, 1], F32, tag=tagp + "sums", name=tagp + "sums")
        nc.scalar.activation(A[:], S[:], AFT.Exp, accum_out=sums[:])
        rec = sm.tile([128, 1], F32, tag=tagp + "rec", name=tagp + "rec")
        nc.vector.reciprocal(rec[:], sums[:])
        nc.vector.tensor_scalar_mul(A[:], A[:], rec[:])
        # --- A^T chunks (PE transpose, DVE copy casts to bf16) ---
        aT = p_at.tile([128, ncols], BF16, tag="aT", name="aT")
        xt_extra = int(os.environ.get("PROBE_XT", "0"))
        for c in range(nch):
            for _x in range(1 + xt_extra):
                ps = p_tp.tile([128, 128], F32, tag="tp", name="tp")
                nc.tensor.matmul(ps[:], A[:, 128 * c:128 * c + 128],
                                 ident[0:128, 0:128], is_transpose=True,
                                 start=True, stop=True)
                nc.vector.tensor_copy(aT[:, 128 * c:128 * c + 128], ps[:])
        # --- AV: per element, V chunk stationary x A^T slice moving ---
        av_extra = int(os.environ.get("PROBE_AV", "0"))
        for j in range(4):
            sl_ = gb + j - b_lo
            for c in range(nch):
                for _x in range(1 + av_extra):
                    nc.tensor.matmul(
                        av_ps[:, 8 * sl_:8 * sl_ + 8],
                        vt[:, j * ncols + 128 * c:j * ncols + 128 * c + 128],
                        aT[:, 128 * c + 32 * j:128 * c + 32 * j + 8],
                        start=(c == 0 and _x == 0), stop=(c == nch - 1),
                        skip_group_check=True,
                    )
            if new_kv is not None:
                nc.tensor.matmul(anew_ps[:, sl_:sl_ + 1],
                                 segT8[32 * j:32 * j + 8, :],
                                 A[32 * j:32 * j + 8, ncols - 1:ncols],
                                 start=(sl_ == 0), stop=True,
                                 tile_position=(32 * j, 0),
                                 skip_group_check=True)
    # --- extraction: attn[d, b] = sum_h av[d, b, h] * seg8[d, h] ---
    tmp = p_x.tile([128, nb * 8], F32, tag="xt", name="xt")
    tv = tmp[:].rearrange("p (b h) -> p b h", h=8)
    av = av_ps[:].rearrange("p (b h) -> p b h", h=8)
    sv = _bc(seg8[:], 1, nb)
    nc.vector.tensor_mul(tv, av, sv)
    nc.vector.tensor_reduce(attn_out, tv, axis=AX.X, op=ALU.add)
    if new_kv is not None:
        v_T = new_kv[1]
        tmp2 = p_x.tile([128, nb], F32, tag="x2", name="x2")
        nc.vector.tensor_mul(tmp2[:], anew_ps[:, 0:nb],
                             v_T[:, b_lo:b_lo + nb])
        nc.vector.tensor_add(attn_out, attn_out, tmp2[:])


# ---------------------------------------------------------------------------
# Host side
# ---------------------------------------------------------------------------

LAST_EXEC_NS = None
LAST_RESULTS = None


def _prep_kv(K, V, ncols, nch):
    """K [B,T,128] -> K^T [128,B,ncols] bf16 (zero-padded);
    V [B,T,128] -> permuted [128(p), B, (c d)] bf16 with
    slot (p, c) = key 128c + p."""
    f32 = np.float32
    bf16 = ml_dtypes.bfloat16
    B, T, D = K.shape
    KT = np.zeros((D, B, ncols), bf16)
    KT[:, :, :T] = np.asarray(K, f32).transpose(2, 0, 1).astype(bf16)
    Vp = np.zeros((B, nch, 128, D), bf16)
    full = (T // 128) if T % 128 else nch
    Vv = np.asarray(V, f32).astype(bf16)
    Vp[:, :full] = Vv[:, :128 * full].reshape(B, full, 128, D)
    if T % 128:
        rem = T - 128 * full
        Vp[:, full, :rem] = Vv[:, 128 * full:]
    # [B, c, p, d] -> [p, B, c, d]
    Vp = np.ascontiguousarray(Vp.transpose(2, 0, 1, 3)).reshape(128, B, -1)
    return KT, Vp


def _host_inputs(h_t, K_att, V_att, K_sa_prev, V_sa_prev, mask,
                 Wq_sa, bq_sa, Wk_sa, bk_sa, Wv_sa, bv_sa, W0_sa, b0_sa,
                 Wq_a, bq_a, W0_a, b0_a, W1, b1, W2, b2,
                 g_sa, be_sa, g_a, be_a, g_mlp, be_mlp):
    f32 = np.float32
    bf16 = ml_dtypes.bfloat16
    qscale = f32(1.0 / np.sqrt(DH))
    h = np.ascontiguousarray(np.asarray(h_t, f32)[:, 0, :])
    maskf = np.full((BSZ, NKP), -1e9, f32)
    maskf[:, :NK] = np.asarray(mask).astype(f32) * f32(-1e9)
    maskf = maskf.astype(bf16)

    common = {
        "Wq_sa": np.asarray(Wq_sa, f32) * qscale,
        "bq_sa": (np.asarray(bq_sa, f32) * qscale).reshape(DIM, 1),
        "Wk_sa": np.asarray(Wk_sa, f32),
        "bk_sa": np.asarray(bk_sa, f32).reshape(DIM, 1),
        "Wv_sa": np.asarray(Wv_sa, f32),
        "bv_sa": np.asarray(bv_sa, f32).reshape(DIM, 1),
        "W0_sa": np.asarray(W0_sa, f32),
        "b0_sa": np.asarray(b0_sa, f32).reshape(DIM, 1),
        "Wq_a": np.asarray(Wq_a, f32) * qscale,
        "bq_a": (np.asarray(bq_a, f32) * qscale).reshape(DIM, 1),
        "W0_a": np.asarray(W0_a, f32),
        "b0_a": np.asarray(b0_a, f32).reshape(DIM, 1),
        "W1": np.asarray(W1, f32),
        "b1": np.asarray(b1, f32).reshape(DIM, 1),
        "W2": np.asarray(W2, f32),
        "b2": np.asarray(b2, f32).reshape(DIM, 1),
        "g_sa": np.asarray(g_sa, f32).reshape(1, DIM),
        "be_sa": np.asarray(be_sa, f32).reshape(1, DIM),
        "g_a": np.asarray(g_a, f32).reshape(1, DIM),
        "be_a": np.asarray(be_a, f32).reshape(1, DIM),
        "g_mlp": np.asarray(g_mlp, f32).reshape(1, DIM),
        "be_mlp": np.asarray(be_mlp, f32).reshape(1, DIM),
        "ident": np.eye(128, dtype=f32),
        "ident_bf": np.eye(128, dtype=f32).astype(bf16),
    }
    seg8 = np.zeros((128, 8), f32)
    for hh in range(NB_HEADS):
        seg8[hh * DH:(hh + 1) * DH, hh] = 1.0
    common["seg8"] = seg8
    segT8 = np.zeros((128, 128), f32)
    for j in range(4):
        segT8[32 * j:32 * j + 8, :] = seg8.T
    common["segT8"] = segT8
    E4 = np.zeros((4, 128), f32)
    for j in range(4):
        E4[j, 32 * j:32 * j + 8] = 1.0
    common["E4"] = E4.astype(bf16)

    KT_att, Vp_att = _prep_kv(np.asarray(K_att, f32), np.asarray(V_att, f32),
                              NKP, NKP // 128)
    KT_sa, Vp_sa = _prep_kv(np.asarray(K_sa_prev, f32),
                            np.asarray(V_sa_prev, f32), TS, TS // 128)

    per_core = []
    Bs = BSZ // N_CORES
    for s in range(N_CORES):
        sl = slice(s * Bs, (s + 1) * Bs)
        m = dict(common)
        m["h_t"] = np.ascontiguousarray(h[sl])
        m["KT_att"] = np.ascontiguousarray(KT_att[:, sl])
        m["Vp_att"] = np.ascontiguousarray(Vp_att[:, sl])
        m["KT_sa"] = np.ascontiguousarray(KT_sa[:, sl])
        m["Vp_sa"] = np.ascontiguousarray(Vp_sa[:, sl])
        m["maskf"] = np.ascontiguousarray(maskf[sl])
        per_core.append(m)
    return per_core


_NC_CACHE = {}


def kernel(**inputs):
    global LAST_EXEC_NS, LAST_RESULTS
    from concourse.bass_utils import run_bass_kernel_spmd

    B = BSZ // N_CORES
    if B not in _NC_CACHE:
        _NC_CACHE[B] = build_nc(B)
    nc = _NC_CACHE[B]
    in_maps = _host_inputs(**inputs)
    trace = os.environ.get("KERNEL_TRACE", "0") == "1"
    res = run_bass_kernel_spmd(nc, in_maps, core_ids=list(range(N_CORES)),
                               trace=trace)
    LAST_EXEC_NS = res.exec_time_ns
    LAST_RESULTS = res
    out = np.concatenate([r["out"] for r in res.results], axis=0)
    return out.astype(np.float32)


# revision 37
# speedup vs baseline: 1.3992x; 1.3992x over previous
"""Trainium2 Bass kernel for nn_AutoRegressiveDecoderLayer.

One transformer decoder step (self-attn with KV cache + masked cross-attn +
MLP, each followed by LayerNorm) over bsz=1024, dim=128, 8 heads.

Strategy: pure data parallel over the batch — 8 NeuronCores, 128 batch
elements each.  Per core everything is expressed on 128-partition tiles.

Key layout decisions (v2):
- The host pre-transposes K to K^T [dim, keys] and pre-casts K and V to
  bf16, stored d-outermost so each per-group DMA moves 8 KB contiguous per
  partition (128 descriptors).  No on-chip K transposes at all.
- V is host-row-permuted so a natural chunked load [p, (c d)] puts key
  t = 128c + p on partition p of chunk c — exactly matching the column
  order of A^T chunks (which are plain PE transposes of the score matrix).
- Self-attn is padded to 512 keys; the fresh key's K^T column is copied
  into kt slot 511 on-chip (plain [128,1] copy in dT layout), so scores
  need no special-casing.  The fresh V row cannot be placed on partition
  127 cheaply, so its AV contribution is added via a tiny per-element
  head-select matmul (anew) + extraction fixup, as in v1.
- Cross-attn is padded to 1024 keys; pad columns get -1e9 in the host
  mask so exp() zeroes them.
- Scores: per batch element, Q-block [128,8] stationary x K^T moving.
  4 elements share a PSUM bank at 32-partition offsets (tile_position).
- Softmax: exp (no max-sub; |scores| small) with fused row-sum, scale.
- A^T via PE transpose per 128-chunk, PSUM->SBUF copy casts to bf16.
- AV: per element per chunk, V-chunk [128,128] bf16 stationary x A^T
  slice [128,8] moving, accumulated in a shared PSUM bank; extracted with
  a block-diagonal mask multiply + reduce into dT layout.
- LayerNorm transposes to batch-major, normalizes with per-partition
  scalars, applies gamma/beta via PE-broadcast tiles, transposes back.
"""

import os

import numpy as np
import ml_dtypes

import concourse.bass as bass
import concourse.bacc as bacc
import concourse.tile as tile
from concourse import mybir

F32 = mybir.dt.float32
BF16 = mybir.dt.bfloat16
AFT = mybir.ActivationFunctionType
AX = mybir.AxisListType
ALU = mybir.AluOpType

DIM = 128
NB_HEADS = 8
DH = DIM // NB_HEADS
N_CORES = 8
BSZ = 1024
NK = 1000   # cross-attention keys (valid)
NKP = 1024  # cross-attention keys (padded)
TP = 511    # self-attn KV cache length (previous)
TS = 512    # self-attn padded length (incl fresh key at slot 511)
LN_EPS = 1e-5

_WNAMES = ["Wq_sa", "Wk_sa", "Wv_sa", "W0_sa", "Wq_a", "W0_a", "W1", "W2"]
_BNAMES = ["bq_sa", "bk_sa", "bv_sa", "b0_sa", "bq_a", "b0_a", "b1", "b2"]
_GNAMES = ["g_sa", "g_a", "g_mlp"]
_BENAMES = ["be_sa", "be_a", "be_mlp"]


def _bc(ap, idx, count):
    """Insert a step-0 (broadcast) dim of `count` at position idx."""
    new = [list(p) for p in ap.ap]
    new.insert(idx, [0, count])
    return bass.AP(ap.tensor, ap.offset, new)


def build_nc(B, reps=1):
    """Build the Bass program for one core processing B batch elements."""
    nc = bacc.Bacc("TRN2", target_bir_lowering=False, debug=False)

    def dpi(name, shape, dt=F32):
        return nc.declare_dram_parameter(name, list(shape), dt, isOutput=False).ap()

    d = {}
    d["h_t"] = dpi("h_t", (B, DIM))
    # host-prepared: K^T d-outermost, V row-permuted p-outermost, both bf16
    d["KT_att"] = dpi("KT_att", (DIM, B, NKP), BF16)
    d["Vp_att"] = dpi("Vp_att", (DIM, B, NKP), BF16)   # [p, b, (c d)]
    d["KT_sa"] = dpi("KT_sa", (DIM, B, TS), BF16)
    d["Vp_sa"] = dpi("Vp_sa", (DIM, B, TS), BF16)      # [p, b, (c d)]
    d["maskf"] = dpi("maskf", (B, NKP), BF16)
    for w in _WNAMES:
        d[w] = dpi(w, (DIM, DIM))
    for b in _BNAMES:
        d[b] = dpi(b, (DIM, 1))
    for g in _GNAMES + _BENAMES:
        d[g] = dpi(g, (1, DIM))
    d["ident"] = dpi("ident", (128, 128))
    d["ident_bf"] = dpi("ident_bf", (128, 128), BF16)
    d["seg8"] = dpi("seg8", (128, 8))
    d["segT8"] = dpi("segT8", (128, 128))
    d["E4"] = dpi("E4", (4, 128), BF16)
    out_h = nc.declare_dram_parameter("out", [B, DIM], F32, isOutput=True).ap()

    with tile.TileContext(nc) as tc:
        for _ in range(reps):
            _emit(nc, tc, d, out_h, B)
    nc.compile()
    return nc


def _emit(nc, tc, d, out_h, B):
    """Emit the full per-core program, pipelined in sub-batches of 64."""
    assert B % 4 == 0
    from contextlib import ExitStack

    SB = min(128, B)
    assert B % SB == 0

    with ExitStack() as ctx:
        # ---------------- pools ----------------
        pers = ctx.enter_context(tc.tile_pool(name="pers", bufs=1))
        sm = ctx.enter_context(tc.tile_pool(name="sm", bufs=3))
        # K/V streaming pools: one buf holds a whole group-of-4's K^T+V.
        # Cross tiles get a deeper ring so prefetch runs ahead through the
        # self-attn phase; self tiles a shallower one (SBUF budget).
        p_kvc = ctx.enter_context(tc.tile_pool(name="kvc", bufs=5))
        p_kvs = ctx.enter_context(tc.tile_pool(name="kvs", bufs=4))
        p_a = ctx.enter_context(tc.tile_pool(name="pa", bufs=2))
        p_at = ctx.enter_context(tc.tile_pool(name="pat", bufs=2))
        p_x = ctx.enter_context(tc.tile_pool(name="px", bufs=2))
        p_mk = ctx.enter_context(tc.tile_pool(name="pmk", bufs=3))
        # PSUM: S_self(1) + S_cross(2) + tp(2) + av(1, per-half) + anew(1)
        # + glue-tp(1) = 8 banks.  The glue gets its own PSUM ring so the
        # per-half linear/LN chain doesn't serialize against the attention
        # pipeline's A^T-transpose ring.
        p_ss = ctx.enter_context(tc.tile_pool(name="pss", bufs=1, space="PSUM"))
        p_sc = ctx.enter_context(tc.tile_pool(name="psc", bufs=1, space="PSUM"))
        p_tp = ctx.enter_context(tc.tile_pool(name="ptp", bufs=2, space="PSUM"))
        p_av = ctx.enter_context(tc.tile_pool(name="pav", bufs=1, space="PSUM"))
        p_gtp = ctx.enter_context(tc.tile_pool(name="gtp", bufs=1, space="PSUM"))
        pools = dict(p_kvc=p_kvc, p_kvs=p_kvs, p_a=p_a, p_at=p_at, p_x=p_x,
                     p_mk=p_mk, p_ss=p_ss, p_sc=p_sc, p_tp=p_tp, p_av=p_av,
                     sm=sm)

        def pt(pool, shape, dtype, tag):
            return pool.tile(list(shape), dtype, tag=tag, name=tag)

        # ---------------- constants / weights ----------------
        # h_t + ident first on the sync queue (the hT->qkv chain gates the
        # first self-attn scores); weights ride the scalar queue.
        h_nat = pt(pers, (B, 128), F32, "h_nat")
        nc.sync.dma_start(h_nat[:], d["h_t"])
        ident = pt(pers, (128, 128), F32, "ident")
        nc.sync.dma_start(ident[:], d["ident"])
        ident_bf = pt(pers, (128, 128), BF16, "ident_bf")
        nc.sync.dma_start(ident_bf[:], d["ident_bf"])
        seg8 = pt(pers, (128, 8), F32, "seg8")
        nc.sync.dma_start(seg8[:], d["seg8"])
        segT8 = pt(pers, (128, 128), F32, "segT8")
        nc.scalar.dma_start(segT8[:], d["segT8"])
        E4 = pt(pers, (4, 128), BF16, "E4")
        nc.sync.dma_start(E4[:], d["E4"])
        zeros4 = pt(pers, (4, 512), BF16, "zeros4")
        nc.vector.memset(zeros4[:], 0.0)

        W = {}
        for w in _WNAMES:
            W[w] = pt(pers, (128, 128), F32, w)
            eng = nc.sync if w in ("Wq_sa", "Wk_sa", "Wv_sa") else nc.scalar
            eng.dma_start(W[w][:], d[w])
        Bi = {}
        for b in _BNAMES:
            Bi[b] = pt(pers, (128, 1), F32, b)
            eng = nc.sync if b in ("bq_sa", "bk_sa", "bv_sa") else nc.scalar
            eng.dma_start(Bi[b][:], d[b])

        # gamma/beta broadcast tiles: ones[1,B].T @ row[1,128] -> [B,128]
        ones1 = pt(pers, (1, B), F32, "ones1")
        nc.vector.memset(ones1[:], 1.0)
        gb_rep = {}
        for nm in _GNAMES + _BENAMES:
            row = pt(pers, (1, 128), F32, "row_" + nm)
            nc.scalar.dma_start(row[:], d[nm])
            ps = pt(p_gtp, (B, 128), F32, "gtp")
            nc.tensor.matmul(ps[:], ones1[:], row[:], start=True, stop=True)
            rep = pt(pers, (B, 128), F32, "rep_" + nm)
            nc.scalar.copy(rep[:], ps[:])
            gb_rep[nm] = rep

        # ---------------- h_t and qkv projections (all B) ----------------
        hT = _transpose_to(nc, p_gtp, pers, h_nat[:], ident, (128, B), "hT")

        def linear(rhs, wname, bname, out_pool, out_tag, func=AFT.Identity,
                   dtype=F32):
            w_ = rhs.free_size()
            ps = pt(p_gtp, (128, w_), F32, "gtp")
            nc.tensor.matmul(ps[:], W[wname][:], rhs, start=True, stop=True)
            out = pt(out_pool, (128, w_), dtype, out_tag)
            nc.scalar.activation(out[:], ps[:], func, bias=Bi[bname][:])
            return out

        q_saT = linear(hT[:], "Wq_sa", "bq_sa", pers, "q_saT")
        k_saT_bf = linear(hT[:], "Wk_sa", "bk_sa", pers, "k_saT_bf", dtype=BF16)
        v_saT = linear(hT[:], "Wv_sa", "bv_sa", pers, "v_saT")

        def q_blk(qT_ap, out, col0, nb):
            ov = out[:, 8 * col0:8 * (col0 + nb)].rearrange(
                "p (b h) -> p b h", h=8)
            qv = _bc(qT_ap, 2, 8)
            sv = _bc(seg8[:], 1, nb)
            nc.vector.tensor_mul(ov, qv, sv)

        Qb_sa = pt(pers, (128, 8 * B), BF16, "Qb_sa")
        q_blk(q_saT[:], Qb_sa, 0, B)

        # ---------------- half-pipelined batch (SB = B = 128) ----------------
        # Attention runs over all 128 rows, but extraction + the linear/LN
        # glue happen per 64-row half as soon as that half's groups finish,
        # so half-1 glue (and the next stage's first groups) overlap half-2
        # attention, and half-1 MLP/out overlaps half-2 cross-attention.
        HF = SB // 2
        halves = [(h0, h0 + HF) for h0 in range(0, SB, HF)]
        attn1 = {h0: pt(sm, (128, HF), F32, f"attn1_{h0}") for h0, _ in halves}
        h1nT = {}
        Qb_a = {}

        def glue1(h0, hf):
            sl = slice(h0, hf)
            t0 = linear(attn1[h0][:], "W0_sa", "b0_sa", sm, f"t0_{h0}")
            h1T = pt(sm, (128, HF), F32, f"h1T_{h0}")
            nc.vector.tensor_add(h1T[:], t0[:], hT[:, sl])
            h1nT[h0] = _layernorm(nc, tc, p_gtp, sm, h1T[:], ident,
                                  gb_rep["g_sa"], gb_rep["be_sa"], h0, HF,
                                  f"h1n_{h0}", out_T=True)
            q_aT = linear(h1nT[h0][:], "Wq_a", "bq_a", sm, f"q_aT_{h0}")
            Qb_a[h0] = pt(sm, (128, 8 * HF), BF16, f"Qb_a_{h0}")
            q_blk(q_aT[:], Qb_a[h0], 0, HF)

        _attention(
            nc, tc, pools, b_lo=0, nb=SB,
            KTsrc=d["KT_sa"], Vsrc=d["Vp_sa"], ncols=TS,
            Qbs=[(0, SB, Qb_sa)], maskf=None, E4=E4, zeros4=zeros4,
            ident=ident, ident_bf=ident_bf, seg8=seg8, segT8=segT8,
            new_kv=(k_saT_bf, v_saT),
            attn_outs=[(h0, hf, attn1[h0]) for h0, hf in halves],
            half_cb=glue1, tagp="s",
        )

        attn2 = {h0: pt(sm, (128, HF), F32, f"attn2_{h0}") for h0, _ in halves}

        def glue2(h0, hf):
            sl = slice(h0, hf)
            t1 = linear(attn2[h0][:], "W0_a", "b0_a", sm, f"t1_{h0}")
            h2T = pt(sm, (128, HF), F32, f"h2T_{h0}")
            nc.vector.tensor_add(h2T[:], t1[:], h1nT[h0][:])
            h2nT = _layernorm(nc, tc, p_gtp, sm, h2T[:], ident,
                              gb_rep["g_a"], gb_rep["be_a"], h0, HF,
                              f"h2n_{h0}", out_T=True)
            mT = linear(h2nT[:], "W1", "b1", sm, f"mT_{h0}", func=AFT.Relu)
            t2 = linear(mT[:], "W2", "b2", sm, f"t2_{h0}")
            h3T = pt(sm, (128, HF), F32, f"h3T_{h0}")
            nc.vector.tensor_add(h3T[:], t2[:], h2nT[:])
            out_nat = _layernorm(nc, tc, p_gtp, sm, h3T[:], ident,
                                 gb_rep["g_mlp"], gb_rep["be_mlp"], h0, HF,
                                 f"h3n_{h0}", out_T=False)
            nc.sync.dma_start(out_h[sl, :], out_nat[:])

        _attention(
            nc, tc, pools, b_lo=0, nb=SB,
            KTsrc=d["KT_att"], Vsrc=d["Vp_att"], ncols=NKP,
            Qbs=[(h0, hf, Qb_a[h0]) for h0, hf in halves],
            maskf=d["maskf"], E4=E4, zeros4=zeros4,
            ident=ident, ident_bf=ident_bf, seg8=seg8, segT8=segT8,
            new_kv=None,
            attn_outs=[(h0, hf, attn2[h0]) for h0, hf in halves],
            half_cb=glue2, tagp="c",
        )


def _transpose_to(nc, p_ps, pool, in_ap, ident, out_shape, tag):
    """PE transpose (fp32) + ACT copy to a new sbuf tile."""
    P, F = in_ap.partition_size(), in_ap.free_size()
    ps = p_ps.tile([F, P], F32, tag="gtp", name="gtp")
    nc.tensor.matmul(ps[:], in_ap, ident[0:P, 0:P], is_transpose=True,
                     start=True, stop=True)
    out = pool.tile(list(out_shape), F32, tag=tag, name=tag)
    nc.scalar.copy(out[:], ps[:])
    return out


def _layernorm(nc, tc, p_tp, sm, xT_ap, ident, g_rep, be_rep, s0, SB, tag,
               out_T):
    """LayerNorm over dim for xT [128(dim), SB]; batch rows s0..s0+SB."""
    nat = _transpose_to(nc, p_tp, sm, xT_ap, ident, (SB, 128), tag + "_nat")
    negmu = sm.tile([SB, 1], F32, tag=tag + "_negmu", name=tag + "_negmu")
    nc.vector.tensor_reduce(negmu[:], nat[:], axis=AX.X, op=ALU.add,
                            negate=True)
    nc.vector.tensor_scalar_mul(negmu[:], negmu[:], 1.0 / DIM)
    cent = sm.tile([SB, 128], F32, tag=tag + "_cent", name=tag + "_cent")
    nc.vector.tensor_scalar_add(cent[:], nat[:], negmu[:])
    sq = sm.tile([SB, 128], F32, tag=tag + "_sq", name=tag + "_sq")
    ssq = sm.tile([SB, 1], F32, tag=tag + "_ssq", name=tag + "_ssq")
    nc.scalar.activation(sq[:], cent[:], AFT.Square, accum_out=ssq[:])
    var = sm.tile([SB, 1], F32, tag=tag + "_var", name=tag + "_var")
    nc.vector.tensor_scalar(var[:], ssq[:], 1.0 / DIM, LN_EPS,
                            op0=ALU.mult, op1=ALU.add)
    sd = sm.tile([SB, 1], F32, tag=tag + "_sd", name=tag + "_sd")
    nc.scalar.activation(sd[:], var[:], AFT.Sqrt)
    rstd = sm.tile([SB, 1], F32, tag=tag + "_rstd", name=tag + "_rstd")
    nc.vector.reciprocal(rstd[:], sd[:])
    nc.vector.tensor_scalar_mul(cent[:], cent[:], rstd[:])
    nc.vector.tensor_mul(cent[:], cent[:], g_rep[0:SB, :])
    nc.vector.tensor_add(cent[:], cent[:], be_rep[0:SB, :])
    if not out_T:
        return cent
    return _transpose_to(nc, p_tp, sm, cent[:], ident, (128, SB), tag + "_T")


def _attention(nc, tc, pools, *, b_lo, nb, KTsrc, Vsrc, ncols, Qbs,
               maskf, E4, zeros4, ident, ident_bf, seg8, segT8, new_kv,
               attn_outs, tagp, half_cb=None):
    """One attention stage for batch rows [b_lo, b_lo+nb), nb <= 128.

    KTsrc: dram AP [128(d), B, ncols] bf16 — pre-transposed K.
    Vsrc:  dram AP [128(p), B, ncols] bf16 — V rows permuted so that
           chunk c, partition p holds key t = 128c + p.
    Scores for 4 batch elements share one PSUM tile at 32-partition
    offsets; softmax is exp (no max-sub) + fused row-sum + scale.
    new_kv: (k_T_bf16 [128,B], v_T_f32 [128,B]) — fresh self-attn K/V.
    The K column is copied into kt slot ncols-1 (so scores include it);
    the V contribution is added via a per-element head-select matmul
    (anew) + extraction fixup, since V slot (127, last) can't be filled
    by a cheap same-partition copy.

    Qbs: [(lo, hi, tile)] — Q-block tiles; element b uses
         tile[:, 8*(b-lo):...].
    attn_outs: [(lo, hi, tile)] — output ranges; extraction runs as soon
         as the groups covering a range complete, then half_cb(lo, hi)
         emits that half's downstream glue (so it can overlap the rest).
    """
    assert nb <= 128 and nb % 4 == 0
    nch = ncols // 128
    banks = [(s, min(512, ncols - s)) for s in range(0, ncols, 512)]

    p_kv = pools["p_kvc"] if maskf is not None else pools["p_kvs"]
    p_a = pools["p_a"]
    p_at = pools["p_at"]
    p_x = pools["p_x"]
    p_mk = pools["p_mk"]
    p_sc = pools["p_ss"] if ncols <= 512 else pools["p_sc"]
    p_tp = pools["p_tp"]
    p_av = pools["p_av"]
    sm = pools["sm"]
    stag = "S_s" if ncols <= 512 else "S_c"

    av_ps = None
    anew_ps = None
    av_lo = 0
    for g in range(nb // 4):
        gb = b_lo + 4 * g
        for (lo_, hi_, _t) in attn_outs:
            if gb == lo_:
                av_lo = lo_
                av_ps = p_av.tile([128, (hi_ - lo_) * 8], F32, tag="av",
                                  name="av")
                if new_kv is not None:
                    anew_ps = p_av.tile([128, hi_ - lo_], F32, tag="anew",
                                        name="anew")
        # --- stream K^T and V for the group of 4 (one DMA each) ---
        kt = p_kv.tile([128, 4 * ncols], BF16, tag="kt" + tagp, name="kt" + tagp)
        # split HWDGE issue: cross-K on SP, self-K on ACT
        keng = nc.sync if maskf is not None else nc.scalar
        for _x in range(1 + int(os.environ.get("PROBE_DMA", "0"))):
            keng.dma_start(
                kt[:].rearrange("d (b t) -> d b t", b=4),
                KTsrc[:, gb:gb + 4, :])
        vt = p_kv.tile([128, 4 * ncols], BF16, tag="vt" + tagp, name="vt" + tagp)
        for _x in range(1 + int(os.environ.get("PROBE_VDMA", "0"))):
            nc.gpsimd.dma_start(
                vt[:].rearrange("p (b t) -> p b t", b=4),
                Vsrc[:, gb:gb + 4, :])
        if new_kv is not None:
            k_T_bf = new_kv[0]
            for j in range(4):
                b = gb + j
                nc.vector.tensor_copy(
                    kt[:, j * ncols + ncols - 1:j * ncols + ncols],
                    k_T_bf[:, b:b + 1])

        S = p_sc.tile([128, ncols], F32, tag=stag, name=stag)
        # --- init: mask rows (cross) or zeros (self), one MM per bank ---
        if maskf is not None:
            mk = p_mk.tile([4, ncols], BF16, tag="mk", name="mk")
            nc.scalar.dma_start(mk[:], maskf[gb:gb + 4, :])
            for (s0_, w) in banks:
                nc.tensor.matmul(S[:, s0_:s0_ + w], E4[:], mk[:, s0_:s0_ + w],
                                 start=True, stop=True, skip_group_check=True)
        else:
            for (s0_, w) in banks:
                nc.tensor.matmul(S[:, s0_:s0_ + w], E4[:], zeros4[:, 0:w],
                                 start=True, stop=True, skip_group_check=True)
        # --- scores ---
        for j in range(4):
            b = gb + j
            qlo, _qhi, qtile = next(e for e in Qbs if e[0] <= b < e[1])
            qb = qtile[:, 8 * (b - qlo):8 * (b - qlo) + 8]
            row = S[32 * j:32 * j + 8, :]
            for (s0_, w) in banks:
                nc.tensor.matmul(row[:, s0_:s0_ + w],
                                 qb, kt[:, j * ncols + s0_:j * ncols + s0_ + w],
                                 start=False, stop=True,
                                 tile_position=(0, 32 * j),
                                 skip_group_check=True)
        # --- softmax: exp + fused row-sum, then reciprocal scale ---
        A = p_a.tile([128, ncols], F32, tag="A", name="A")
        sums = sm.tile([128, 1], F32, tag=tagp + "sums", name=tagp + "sums")
        nc.scalar.activation(A[:], S[:], AFT.Exp, accum_out=sums[:])
        rec = sm.tile([128, 1], F32, tag=tagp + "rec", name=tagp + "rec")
        nc.vector.reciprocal(rec[:], sums[:])
        nc.vector.tensor_scalar_mul(A[:], A[:], rec[:])
        # --- A^T chunks (PE transpose, DVE copy casts to bf16) ---
        aT = p_at.tile([128, ncols], BF16, tag="aT", name="aT")
        xt_extra = int(os.environ.get("PROBE_XT", "0"))
        for c in range(nch):
            for _x in range(1 + xt_extra):
                ps = p_tp.tile([128, 128], F32, tag="tp", name="tp")
                nc.tensor.matmul(ps[:], A[:, 128 * c:128 * c + 128],
                                 ident[0:128, 0:128], is_transpose=True,
                                 start=True, stop=True)
                nc.vector.tensor_copy(aT[:, 128 * c:128 * c + 128], ps[:])
        # --- AV: per element, V chunk stationary x A^T slice moving ---
        av_extra = int(os.environ.get("PROBE_AV", "0"))
        for j in range(4):
            sl_ = gb + j - av_lo
            for c in range(nch):
                for _x in range(1 + av_extra):
                    nc.tensor.matmul(
                        av_ps[:, 8 * sl_:8 * sl_ + 8],
                        vt[:, j * ncols + 128 * c:j * ncols + 128 * c + 128],
                        aT[:, 128 * c + 32 * j:128 * c + 32 * j + 8],
                        start=(c == 0 and _x == 0), stop=(c == nch - 1),
                        skip_group_check=True,
                    )
            if new_kv is not None:
                nc.tensor.matmul(anew_ps[:, sl_:sl_ + 1],
                                 segT8[32 * j:32 * j + 8, :],
                                 A[32 * j:32 * j + 8, ncols - 1:ncols],
                                 start=(sl_ == 0), stop=True,
                                 tile_position=(32 * j, 0),
                                 skip_group_check=True)
        # --- per-range extraction as soon as its groups are done ---
        b_end = gb + 4
        for (lo, hi, out_t) in attn_outs:
            if b_end != hi:
                continue
            w_ = hi - lo
            tmp = p_x.tile([128, w_ * 8], F32, tag="xt", name="xt")
            tv = tmp[:].rearrange("p (b h) -> p b h", h=8)
            av = av_ps[:, 0:8 * (hi - lo)].rearrange(
                "p (b h) -> p b h", h=8)
            sv = _bc(seg8[:], 1, w_)
            nc.vector.tensor_mul(tv, av, sv)
            nc.vector.tensor_reduce(out_t[:], tv, axis=AX.X, op=ALU.add)
            if new_kv is not None:
                v_T = new_kv[1]
                tmp2 = p_x.tile([128, w_], F32, tag="x2", name="x2")
                nc.vector.tensor_mul(tmp2[:],
                                     anew_ps[:, 0:hi - lo],
                                     v_T[:, lo:hi])
                nc.vector.tensor_add(out_t[:], out_t[:], tmp2[:])
            if half_cb is not None:
                half_cb(lo, hi)


# ---------------------------------------------------------------------------
# Host side
# ---------------------------------------------------------------------------

LAST_EXEC_NS = None
LAST_RESULTS = None


def _prep_kv(K, V, ncols, nch):
    """K [B,T,128] -> K^T [128,B,ncols] bf16 (zero-padded);
    V [B,T,128] -> permuted [128(p), B, (c d)] bf16 with
    slot (p, c) = key 128c + p."""
    f32 = np.float32
    bf16 = ml_dtypes.bfloat16
    B, T, D = K.shape
    KT = np.zeros((D, B, ncols), bf16)
    KT[:, :, :T] = np.asarray(K, f32).transpose(2, 0, 1).astype(bf16)
    Vp = np.zeros((B, nch, 128, D), bf16)
    full = (T // 128) if T % 128 else nch
    Vv = np.asarray(V, f32).astype(bf16)
    Vp[:, :full] = Vv[:, :128 * full].reshape(B, full, 128, D)
    if T % 128:
        rem = T - 128 * full
        Vp[:, full, :rem] = Vv[:, 128 * full:]
    # [B, c, p, d] -> [p, B, c, d]
    Vp = np.ascontiguousarray(Vp.transpose(2, 0, 1, 3)).reshape(128, B, -1)
    return KT, Vp


def _host_inputs(h_t, K_att, V_att, K_sa_prev, V_sa_prev, mask,
                 Wq_sa, bq_sa, Wk_sa, bk_sa, Wv_sa, bv_sa, W0_sa, b0_sa,
                 Wq_a, bq_a, W0_a, b0_a, W1, b1, W2, b2,
                 g_sa, be_sa, g_a, be_a, g_mlp, be_mlp):
    f32 = np.float32
    bf16 = ml_dtypes.bfloat16
    qscale = f32(1.0 / np.sqrt(DH))
    h = np.ascontiguousarray(np.asarray(h_t, f32)[:, 0, :])
    maskf = np.full((BSZ, NKP), -1e9, f32)
    maskf[:, :NK] = np.asarray(mask).astype(f32) * f32(-1e9)
    maskf = maskf.astype(bf16)

    common = {
        "Wq_sa": np.asarray(Wq_sa, f32) * qscale,
        "bq_sa": (np.asarray(bq_sa, f32) * qscale).reshape(DIM, 1),
        "Wk_sa": np.asarray(Wk_sa, f32),
        "bk_sa": np.asarray(bk_sa, f32).reshape(DIM, 1),
        "Wv_sa": np.asarray(Wv_sa, f32),
        "bv_sa": np.asarray(bv_sa, f32).reshape(DIM, 1),
        "W0_sa": np.asarray(W0_sa, f32),
        "b0_sa": np.asarray(b0_sa, f32).reshape(DIM, 1),
        "Wq_a": np.asarray(Wq_a, f32) * qscale,
        "bq_a": (np.asarray(bq_a, f32) * qscale).reshape(DIM, 1),
        "W0_a": np.asarray(W0_a, f32),
        "b0_a": np.asarray(b0_a, f32).reshape(DIM, 1),
        "W1": np.asarray(W1, f32),
        "b1": np.asarray(b1, f32).reshape(DIM, 1),
        "W2": np.asarray(W2, f32),
        "b2": np.asarray(b2, f32).reshape(DIM, 1),
        "g_sa": np.asarray(g_sa, f32).reshape(1, DIM),
        "be_sa": np.asarray(be_sa, f32).reshape(1, DIM),
        "g_a": np.asarray(g_a, f32).reshape(1, DIM),
        "be_a": np.asarray(be_a, f32).reshape(1, DIM),
        "g_mlp": np.asarray(g_mlp, f32).reshape(1, DIM),
        "be_mlp": np.asarray(be_mlp, f32).reshape(1, DIM),
        "ident": np.eye(128, dtype=f32),
        "ident_bf": np.eye(128, dtype=f32).astype(bf16),
    }
    seg8 = np.zeros((128, 8), f32)
    for hh in range(NB_HEADS):
        seg8[hh * DH:(hh + 1) * DH, hh] = 1.0
    common["seg8"] = seg8
    segT8 = np.zeros((128, 128), f32)
    for j in range(4):
        segT8[32 * j:32 * j + 8, :] = seg8.T
    common["segT8"] = segT8
    E4 = np.zeros((4, 128), f32)
    for j in range(4):
        E4[j, 32 * j:32 * j + 8] = 1.0
    common["E4"] = E4.astype(bf16)

    KT_att, Vp_att = _prep_kv(np.asarray(K_att, f32), np.asarray(V_att, f32),
                              NKP, NKP // 128)
    KT_sa, Vp_sa = _prep_kv(np.asarray(K_sa_prev, f32),
                            np.asarray(V_sa_prev, f32), TS, TS // 128)

    per_core = []
    Bs = BSZ // N_CORES
    for s in range(N_CORES):
        sl = slice(s * Bs, (s + 1) * Bs)
        m = dict(common)
        m["h_t"] = np.ascontiguousarray(h[sl])
        m["KT_att"] = np.ascontiguousarray(KT_att[:, sl])
        m["Vp_att"] = np.ascontiguousarray(Vp_att[:, sl])
        m["KT_sa"] = np.ascontiguousarray(KT_sa[:, sl])
        m["Vp_sa"] = np.ascontiguousarray(Vp_sa[:, sl])
        m["maskf"] = np.ascontiguousarray(maskf[sl])
        per_core.append(m)
    return per_core


_NC_CACHE = {}


def kernel(**inputs):
    global LAST_EXEC_NS, LAST_RESULTS
    from concourse.bass_utils import run_bass_kernel_spmd

    B = BSZ // N_CORES
    if B not in _NC_CACHE:
        _NC_CACHE[B] = build_nc(B)
    nc = _NC_CACHE[B]
    in_maps = _host_inputs(**inputs)
    trace = os.environ.get("KERNEL_TRACE", "0") == "1"
    res = run_bass_kernel_spmd(nc, in_maps, core_ids=list(range(N_CORES)),
                               trace=trace)
    LAST_EXEC_NS = res.exec_time_ns
    LAST_RESULTS = res
    out = np.concatenate([r["out"] for r in res.results], axis=0)
    return out.astype(np.float32)
